# revision 8
# baseline (speedup 1.0000x reference)
"""Single-head causal attention (B=8, T=2048, E=1024, H=64) on 8 trn2 cores.

Sharding: data-parallel over batch — core b handles batch element b.

Per-core dataflow (everything kept transposed until the very end):
  x [T,E] f32 --cast DMA--> x_nat f16 --PE transpose--> xT [E,T] f16
  qkT psum = [Wq|Wk]^T @ xT   (packed: q rows 0-63, k rows 64-127)
  vT  psum = Wv^T @ xT
  kT copied out of qkT via SBUF->SBUF DMA (partition shift)
  v1 [s,65] = PE-transpose(vT) with a ones column appended (rowsum trick)
  scoresT[s,t] psum = kT_blk^T @ qT   (K=64), causal: only t >= s blocks
  wT = exp(0.125 * scoresT) on ACT (PSUM->SBUF, f16), diag block masked
  outT[65,t] psum += v1_j^T @ wT_j    (row 64 accumulates the softmax denom)
  out[t,h] = PE-transpose(outT) / denom, stored f32
"""

import numpy as np

import concourse.bass as bass
import concourse.mybir as mybir
from concourse.tile import TileContext
from concourse.masks import make_identity, make_upper_triangular
from concourse.bass_utils import run_bass_kernel_spmd

B, T, E, H = 8, 2048, 1024, 64
NT = T // 128   # 16 t-tiles
NE = E // 128   # 8 e-tiles
F16 = mybir.dt.float16
F32 = mybir.dt.float32
SCALE = float(H) ** -0.5  # 0.125

def _split_excess_waits(nc: bass.Bass, cap: int = 1) -> int:
    """This walrus build allows only `cap` sync-waits per instruction
    ("Too many sync wait commands" in CoreV3GenImpl otherwise), while Tile's
    wait assignment attaches one wait per producer proc. Hoist the extra
    waits onto NOPs inserted just before the instruction on the same engine:
    the engine blocks on each NOP's wait first, so the semantics of
    "all waits hold before the instruction executes" are preserved."""
    n_split = 0
    for f in nc.m.functions:
        for bb in f.blocks:
            insts = list(bb.instructions)
            out = []
            dirty = False
            for inst in insts:
                si = inst.sync_info
                waits = list(si.on_wait) if si and si.on_wait else []
                if len(waits) > cap:
                    si.on_wait = waits[:cap]
                    for w in waits[cap:]:
                        nop = mybir.InstNoOp(
                            name=f"I-waitsplit-{n_split}", ins=[], outs=[]
                        )
                        nop.engine = inst.engine
                        nop.sync_info = mybir.SyncInfo(on_wait=[w], on_update=[])
                        out.append(nop)
                        n_split += 1
                    dirty = True
                out.append(inst)
            if dirty:
                bb.instructions = out
    return n_split


def build_nc(split_waits: bool = True) -> bass.Bass:
    nc = bass.Bass()
    x = nc.dram_tensor("x", [T, E], F32, kind="ExternalInput")
    wq = nc.dram_tensor("Wq", [E, H], F32, kind="ExternalInput")
    wk = nc.dram_tensor("Wk", [E, H], F32, kind="ExternalInput")
    wv = nc.dram_tensor("Wv", [E, H], F32, kind="ExternalInput")
    out = nc.dram_tensor("out", [T, H], F32, kind="ExternalOutput")
    x_ap, out_ap = x.ap(), out.ap()

    with TileContext(nc) as tc:
        with (
            tc.tile_pool(name="const", bufs=1) as cpool,
            tc.tile_pool(name="wts", bufs=8) as wpool,
            tc.tile_pool(name="xnat", bufs=8) as xnpool,
            tc.tile_pool(name="xT", bufs=8) as xtpool,
            tc.tile_pool(name="qkv", bufs=1) as qkvpool,
            tc.tile_pool(name="wT", bufs=3) as wtpool,
            tc.tile_pool(name="fin", bufs=4) as finpool,
        ):
            eye16 = cpool.tile([128, 128], F16, tag="eye16")
            make_identity(nc, eye16[:])
            eye32 = cpool.tile([128, 128], F32, tag="eye32")
            make_identity(nc, eye32[:])
            # tri[s,t] = 1 where s <= t (valid causal region of a diag block)
            tri = cpool.tile([128, 128], F16, tag="tri")
            make_upper_triangular(nc, tri[:], val=1.0, diag=True)

            wqk_t = []
            wv_t = []
            for j in range(NE):
                wt = wpool.tile([128, 128], F16, tag="wqk")
                nc.gpsimd.dma_start(wt[:, 0:64], wq.ap()[128 * j : 128 * j + 128, :])
                nc.gpsimd.dma_start(wt[:, 64:128], wk.ap()[128 * j : 128 * j + 128, :])
                wqk_t.append(wt)
                vt = wpool.tile([128, 64], F16, tag="wv")
                nc.gpsimd.dma_start(vt[:], wv.ap()[128 * j : 128 * j + 128, :])
                wv_t.append(vt)

            qkT = qkvpool.tile([128, T], F16, tag="qkT")  # q rows 0-63, k rows 64-127
            kT = qkvpool.tile([64, T], F16, tag="kT")
            vT = qkvpool.tile([64, T], F16, tag="vT")
            v1 = qkvpool.tile([128, NT * 65], F16, tag="v1")
            outT_sb = qkvpool.tile([65, T], F32, tag="outT")

            xT = [
                xtpool.tile([128, T], F16, tag="xT", name=f"xT{j}")
                for j in range(NE)
            ]

            # ---------- phase A: load x, transpose to xT, projections ----------
            with tc.tile_pool(name="ps1", bufs=1, space="PSUM") as ps1:
                xn = []
                for i in range(NT):
                    t_ = xnpool.tile([128, E], F16, tag="xn")
                    nc.gpsimd.dma_start(t_[:], x_ap[128 * i : 128 * i + 128, :])
                    xn.append(t_)

                # transpose 4 t-tiles at a time into one psum bank per e-tile
                for i0 in range(0, NT, 4):
                    for j in range(NE):
                        pt = ps1.tile([128, 512], F16, tag="tp", bufs=3)
                        for ii in range(4):
                            nc.tensor.transpose(
                                pt[:, 128 * ii : 128 * ii + 128],
                                xn[i0 + ii][:, 128 * j : 128 * j + 128],
                                eye16[:],
                            )
                        nc.vector.tensor_copy(
                            xT[j][:, 128 * i0 : 128 * i0 + 512], pt[:]
                        )

                # projections: psum_qk[m, t] with q in rows 0-63, k in 64-127
                for c in range(4):
                    sl = slice(512 * c, 512 * c + 512)
                    pqk = ps1.tile([128, 512], F32, tag="pqk", bufs=2)
                    pv = ps1.tile([64, 512], F32, tag="pv", bufs=2)
                    for j in range(NE):
                        nc.tensor.matmul(
                            pqk[:], wqk_t[j][:], xT[j][:, sl],
                            start=(j == 0), stop=(j == NE - 1),
                        )
                        nc.tensor.matmul(
                            pv[:], wv_t[j][:], xT[j][:, sl],
                            start=(j == 0), stop=(j == NE - 1),
                        )
                    nc.vector.tensor_copy(qkT[:, sl], pqk[:])
                    nc.scalar.copy(vT[:, sl], pv[:])
                    # kT = k rows of qkT moved to partitions 0-63
                    nc.sync.dma_start(kT[:, sl], qkT[64:128, sl])

            # ---------- phase B: v1, attention, finalize ----------
            with tc.tile_pool(name="ps2", bufs=1, space="PSUM") as ps2:
                for i in range(NT):
                    pt = ps2.tile([128, 64], F16, tag="sc", bufs=2)
                    nc.tensor.transpose(
                        pt[:, 0:64], vT[:, 128 * i : 128 * i + 128], eye16[0:64, 0:64]
                    )
                    nc.vector.tensor_copy(v1[:, 65 * i : 65 * i + 64], pt[:, 0:64])
                    nc.gpsimd.memset(v1[:, 65 * i + 64 : 65 * i + 65], 1.0)

                outT_ps = [
                    ps2.tile([65, 512], F32, tag="ot", bufs=4, name=f"ot{c}")
                    for c in range(4)
                ]
                for j in range(NT):
                    s0 = 128 * j  # this s-tile covers s in [s0, s0+128)
                    Lj = T - s0
                    wT = wtpool.tile([128, Lj], F16, tag="wT")
                    # scoresT in [128,<=1024] psum tiles on absolute-1024 grid
                    for kc in range(s0 // 1024, 2):
                        t0 = max(1024 * kc, s0)
                        t1 = 1024 * kc + 1024
                        w = t1 - t0
                        ps = ps2.tile([128, w], F32, tag="sc", bufs=2)
                        off = 0
                        while off < w:
                            n = min(512, w - off)
                            nc.tensor.matmul(
                                ps[:, off : off + n],
                                kT[:, s0 : s0 + 128],
                                qkT[0:64, t0 + off : t0 + off + n],
                                start=True, stop=True,
                            )
                            off += n
                        nc.scalar.activation(
                            wT[:, t0 - s0 : t1 - s0], ps[:],
                            mybir.ActivationFunctionType.Exp, scale=SCALE,
                        )
                    # mask the diagonal block: keep only s <= t
                    nc.vector.tensor_mul(wT[:, 0:128], wT[:, 0:128], tri[:])
                    # outT[65, t] += v1_j^T @ wT_j  per absolute-512 chunk
                    for c in range(s0 // 512, 4):
                        t0 = max(512 * c, s0)
                        t1 = 512 * c + 512
                        pad = t0 - 512 * c
                        nc.tensor.matmul(
                            outT_ps[c][:, pad:512],
                            v1[:, 65 * j : 65 * j + 65],
                            wT[:, t0 - s0 : t1 - s0],
                            start=(j == 0), stop=(j == 4 * c + 3),
                        )
                for c in range(4):
                    nc.vector.tensor_copy(
                        outT_sb[:, 512 * c : 512 * c + 512], outT_ps[c][:]
                    )

                for i in range(NT):
                    pt = ps2.tile([128, 65], F32, tag="sc", bufs=2)
                    nc.tensor.transpose(
                        pt[:, 0:65],
                        outT_sb[:, 128 * i : 128 * i + 128],
                        eye32[0:65, 0:65],
                    )
                    rcp = finpool.tile([128, 1], F32, tag="rcp")
                    nc.vector.reciprocal(rcp[:], pt[:, 64:65])
                    ob = finpool.tile([128, 64], F32, tag="ob")
                    nc.vector.tensor_scalar_mul(ob[:], pt[:, 0:64], rcp[:])
                    nc.sync.dma_start(out_ap[128 * i : 128 * i + 128, :], ob[:])

    if split_waits:
        # required for walrus codegen; CoreSim rejects the raw NOPs, so
        # sim runs build with split_waits=False
        _split_excess_waits(nc)
    return nc


_NC = None


def _get_nc() -> bass.Bass:
    global _NC
    if _NC is None:
        _NC = build_nc()
    return _NC


def kernel(x, Wq, Wk, Wv, **run_kwargs):
    nc = _get_nc()
    x = np.ascontiguousarray(x, dtype=np.float32)
    in_maps = [
        {
            "x": np.ascontiguousarray(x[b]),
            "Wq": np.ascontiguousarray(Wq, dtype=np.float32),
            "Wk": np.ascontiguousarray(Wk, dtype=np.float32),
            "Wv": np.ascontiguousarray(Wv, dtype=np.float32),
        }
        for b in range(B)
    ]
    res = run_bass_kernel_spmd(nc, in_maps, core_ids=list(range(B)), **run_kwargs)
    out = np.stack([res.results[b]["out"] for b in range(B)], axis=0)
    kernel.last_results = res
    return out


# revision 12
# speedup vs baseline: 1.0582x; 1.0582x over previous
"""Single-head causal attention (B=8, T=2048, E=1024, H=64) on 8 trn2 cores.

Sharding: data-parallel over batch — core b handles batch element b.

Per-core dataflow (everything kept transposed until the very end):
  x [T,E] f32 --cast DMA--> x_nat f16 --PE transpose--> xT [E,T] f16
  qkT psum = [Wq|Wk]^T @ xT   (packed: q rows 0-63, k rows 64-127)
  vT  psum = Wv^T @ xT
  kT copied out of qkT via SBUF->SBUF DMA (partition shift)
  v1 [s,65] = PE-transpose(vT) with a ones column appended (rowsum trick)
  scoresT[s,t] psum = kT_blk^T @ qT   (K=64), causal: only t >= s blocks
  wT = exp(0.125 * scoresT) on ACT (PSUM->SBUF, f16), diag block masked
  outT[65,t] psum += v1_j^T @ wT_j    (row 64 accumulates the softmax denom)
  out[t,h] = PE-transpose(outT) / denom, stored f32
"""

import numpy as np

import concourse.bass as bass
import concourse.mybir as mybir
from concourse.tile import TileContext
from concourse.masks import make_identity, make_upper_triangular
from concourse.bass_utils import run_bass_kernel_spmd

B, T, E, H = 8, 2048, 1024, 64
NT = T // 128   # 16 t-tiles
NE = E // 128   # 8 e-tiles
F16 = mybir.dt.float16
F32 = mybir.dt.float32
SCALE = float(H) ** -0.5  # 0.125

def _split_excess_waits(nc: bass.Bass, cap: int = 1) -> int:
    """This walrus build allows only `cap` sync-waits per instruction
    ("Too many sync wait commands" in CoreV3GenImpl otherwise), while Tile's
    wait assignment attaches one wait per producer proc. Hoist the extra
    waits onto NOPs inserted just before the instruction on the same engine:
    the engine blocks on each NOP's wait first, so the semantics of
    "all waits hold before the instruction executes" are preserved."""
    n_split = 0
    for f in nc.m.functions:
        for bb in f.blocks:
            insts = list(bb.instructions)
            out = []
            dirty = False
            for inst in insts:
                si = inst.sync_info
                waits = list(si.on_wait) if si and si.on_wait else []
                if len(waits) > cap:
                    si.on_wait = waits[:cap]
                    for w in waits[cap:]:
                        nop = mybir.InstNoOp(
                            name=f"I-waitsplit-{n_split}", ins=[], outs=[]
                        )
                        nop.engine = inst.engine
                        nop.sync_info = mybir.SyncInfo(on_wait=[w], on_update=[])
                        out.append(nop)
                        n_split += 1
                    dirty = True
                out.append(inst)
            if dirty:
                bb.instructions = out
    return n_split


def build_nc(split_waits: bool = True) -> bass.Bass:
    nc = bass.Bass()
    x = nc.dram_tensor("x", [T, E], F32, kind="ExternalInput")
    wq = nc.dram_tensor("Wq", [E, H], F32, kind="ExternalInput")
    wk = nc.dram_tensor("Wk", [E, H], F32, kind="ExternalInput")
    wv = nc.dram_tensor("Wv", [E, H], F32, kind="ExternalInput")
    out = nc.dram_tensor("out", [T, H], F32, kind="ExternalOutput")
    x_ap, out_ap = x.ap(), out.ap()

    with TileContext(nc) as tc:
        with (
            tc.tile_pool(name="const", bufs=1) as cpool,
            tc.tile_pool(name="wts", bufs=8) as wpool,
            tc.tile_pool(name="xnat", bufs=8) as xnpool,
            tc.tile_pool(name="xT", bufs=8) as xtpool,
            tc.tile_pool(name="qkv", bufs=1) as qkvpool,
            tc.tile_pool(name="wT", bufs=3) as wtpool,
            tc.tile_pool(name="fin", bufs=4) as finpool,
        ):
            eye16 = cpool.tile([128, 128], F16, tag="eye16")
            make_identity(nc, eye16[:])
            eye32 = cpool.tile([128, 128], F32, tag="eye32")
            make_identity(nc, eye32[:])
            # tri[s,t] = 1 where s <= t (valid causal region of a diag block)
            tri = cpool.tile([128, 128], F16, tag="tri")
            make_upper_triangular(nc, tri[:], val=1.0, diag=True)

            w32 = {}
            for nm, dram in (("q", wq), ("k", wk), ("v", wv)):
                t32 = wpool.tile(
                    [128, NE * H], F32, tag="w32", bufs=3, name=f"w32{nm}"
                )
                nc.sync.dma_start(
                    t32[:].rearrange("p (j h) -> p j h", h=H),
                    dram.ap().rearrange("(j p) h -> p j h", p=128),
                )
                w32[nm] = t32
            wqk_t = []
            wv_t = []
            for j in range(NE):
                wt = wpool.tile([128, 128], F16, tag="wqk")
                nc.vector.tensor_copy(wt[:, 0:64], w32["q"][:, H * j : H * j + H])
                nc.vector.tensor_copy(wt[:, 64:128], w32["k"][:, H * j : H * j + H])
                wqk_t.append(wt)
                vt = wpool.tile([128, 64], F16, tag="wv")
                nc.vector.tensor_copy(vt[:], w32["v"][:, H * j : H * j + H])
                wv_t.append(vt)

            qkT = qkvpool.tile([128, T], F16, tag="qkT")  # q rows 0-63, k rows 64-127
            kT = qkvpool.tile([64, T], F16, tag="kT")
            vT = qkvpool.tile([64, T], F16, tag="vT")
            v1 = qkvpool.tile([128, NT * 65], F16, tag="v1")
            outT_sb = qkvpool.tile([65, T], F32, tag="outT")

            xT = [
                xtpool.tile([128, T], F16, tag="xT", name=f"xT{j}")
                for j in range(NE)
            ]

            # ---------- phase A: load x, transpose to xT, projections ----------
            with tc.tile_pool(name="ps1", bufs=1, space="PSUM") as ps1:
                x32s = []
                for i in range(NT):
                    t32 = xnpool.tile([128, E], F32, tag="x32", bufs=6, name=f"x32_{i}")
                    nc.sync.dma_start(t32[:], x_ap[128 * i : 128 * i + 128, :])
                    x32s.append(t32)
                xn = []
                for i in range(NT):
                    t_ = xnpool.tile([128, E], F16, tag="xn", bufs=8, name=f"xn{i}")
                    nc.gpsimd.tensor_copy(t_[:], x32s[i][:])
                    xn.append(t_)

                # transpose 4 t-tiles at a time into one psum bank per e-tile
                for i0 in range(0, NT, 4):
                    for j in range(NE):
                        pt = ps1.tile([128, 512], F32, tag="tp", bufs=3)
                        for ii in range(4):
                            nc.tensor.matmul(
                                pt[:, 128 * ii : 128 * ii + 128],
                                xn[i0 + ii][:, 128 * j : 128 * j + 128],
                                eye16[:],
                                start=True, stop=True,
                            )
                        nc.vector.tensor_copy(
                            xT[j][:, 128 * i0 : 128 * i0 + 512], pt[:]
                        )

                # projections: psum_qk[m, t] with q in rows 0-63, k in 64-127
                for c in range(4):
                    sl = slice(512 * c, 512 * c + 512)
                    pqk = ps1.tile([128, 512], F32, tag="pqk", bufs=2)
                    pv = ps1.tile([64, 512], F32, tag="pv", bufs=2)
                    for j in range(NE):
                        nc.tensor.matmul(
                            pqk[:], wqk_t[j][:], xT[j][:, sl],
                            start=(j == 0), stop=(j == NE - 1),
                        )
                        nc.tensor.matmul(
                            pv[:], wv_t[j][:], xT[j][:, sl],
                            start=(j == 0), stop=(j == NE - 1),
                        )
                    nc.vector.tensor_copy(qkT[:, sl], pqk[:])
                    nc.scalar.copy(vT[:, sl], pv[:])
                    # kT = k rows of qkT moved to partitions 0-63
                    nc.sync.dma_start(kT[:, sl], qkT[64:128, sl])

            # ---------- phase B: v1, attention, finalize ----------
            with tc.tile_pool(name="ps2", bufs=1, space="PSUM") as ps2:
                for i in range(NT):
                    pt = ps2.tile([128, 64], F32, tag="sc", bufs=2)
                    nc.tensor.matmul(
                        pt[:, 0:64], vT[:, 128 * i : 128 * i + 128],
                        eye16[0:64, 0:64], start=True, stop=True,
                    )
                    nc.vector.tensor_copy(v1[:, 65 * i : 65 * i + 64], pt[:, 0:64])
                    nc.gpsimd.memset(v1[:, 65 * i + 64 : 65 * i + 65], 1.0)

                outT_ps = [
                    ps2.tile([65, 512], F32, tag="ot", bufs=4, name=f"ot{c}")
                    for c in range(4)
                ]
                wTs = {}

                def emit_2ndmm(j):
                    # outT[65, t] += v1_j^T @ wT_j  per absolute-512 chunk
                    s0 = 128 * j
                    for c in range(s0 // 512, 4):
                        t0 = max(512 * c, s0)
                        t1 = 512 * c + 512
                        pad = t0 - 512 * c
                        nc.tensor.matmul(
                            outT_ps[c][:, pad:512],
                            v1[:, 65 * j : 65 * j + 65],
                            wTs[j][:, t0 - s0 : t1 - s0],
                            start=(j == 0), stop=(j == 4 * c + 3),
                        )

                for j in range(NT):
                    s0 = 128 * j  # this s-tile covers s in [s0, s0+128)
                    Lj = T - s0
                    wT = wtpool.tile([128, Lj], F16, tag="wT", bufs=4, name=f"wT{j}")
                    wTs[j] = wT
                    # scoresT in [128,<=1024] psum tiles on absolute-1024 grid
                    for kc in range(s0 // 1024, 2):
                        t0 = max(1024 * kc, s0)
                        t1 = 1024 * kc + 1024
                        w = t1 - t0
                        ps = ps2.tile([128, w], F32, tag="sc", bufs=2)
                        off = 0
                        while off < w:
                            n = min(512, w - off)
                            nc.tensor.matmul(
                                ps[:, off : off + n],
                                kT[:, s0 : s0 + 128],
                                qkT[0:64, t0 + off : t0 + off + n],
                                start=True, stop=True,
                            )
                            off += n
                        nc.scalar.activation(
                            wT[:, t0 - s0 : t1 - s0], ps[:],
                            mybir.ActivationFunctionType.Exp, scale=SCALE,
                        )
                    # mask the diagonal block: keep only s <= t
                    nc.vector.tensor_mul(wT[:, 0:128], wT[:, 0:128], tri[:])
                    # 2nd matmul lags one s-tile so PE never waits on exp_j
                    if j >= 1:
                        emit_2ndmm(j - 1)
                emit_2ndmm(NT - 1)
                for c in range(4):
                    nc.vector.tensor_copy(
                        outT_sb[:, 512 * c : 512 * c + 512], outT_ps[c][:]
                    )

                for i in range(NT):
                    pt = ps2.tile([128, 65], F32, tag="sc", bufs=2)
                    nc.tensor.transpose(
                        pt[:, 0:65],
                        outT_sb[:, 128 * i : 128 * i + 128],
                        eye32[0:65, 0:65],
                    )
                    rcp = finpool.tile([128, 1], F32, tag="rcp")
                    nc.vector.reciprocal(rcp[:], pt[:, 64:65])
                    ob = finpool.tile([128, 64], F32, tag="ob")
                    nc.vector.tensor_scalar_mul(ob[:], pt[:, 0:64], rcp[:])
                    nc.sync.dma_start(out_ap[128 * i : 128 * i + 128, :], ob[:])

    if split_waits:
        # required for walrus codegen; CoreSim rejects the raw NOPs, so
        # sim runs build with split_waits=False
        _split_excess_waits(nc)
    return nc


_NC = None


def _get_nc() -> bass.Bass:
    global _NC
    if _NC is None:
        _NC = build_nc()
    return _NC


def kernel(x, Wq, Wk, Wv, **run_kwargs):
    nc = _get_nc()
    x = np.ascontiguousarray(x, dtype=np.float32)
    in_maps = [
        {
            "x": np.ascontiguousarray(x[b]),
            "Wq": np.ascontiguousarray(Wq, dtype=np.float32),
            "Wk": np.ascontiguousarray(Wk, dtype=np.float32),
            "Wv": np.ascontiguousarray(Wv, dtype=np.float32),
        }
        for b in range(B)
    ]
    res = run_bass_kernel_spmd(nc, in_maps, core_ids=list(range(B)), **run_kwargs)
    out = np.stack([res.results[b]["out"] for b in range(B)], axis=0)
    kernel.last_results = res
    return out


# revision 13
# speedup vs baseline: 1.3245x; 1.2517x over previous
"""Single-head causal attention (B=8, T=2048, E=1024, H=64) on 8 trn2 cores.

Sharding: data-parallel over batch — core b handles batch element b.

Per-core dataflow (everything kept transposed until the very end):
  x [T,E] f32 --cast DMA--> x_nat f16 --PE transpose--> xT [E,T] f16
  qkT psum = [Wq|Wk]^T @ xT   (packed: q rows 0-63, k rows 64-127)
  vT  psum = Wv^T @ xT
  kT copied out of qkT via SBUF->SBUF DMA (partition shift)
  v1 [s,65] = PE-transpose(vT) with a ones column appended (rowsum trick)
  scoresT[s,t] psum = kT_blk^T @ qT   (K=64), causal: only t >= s blocks
  wT = exp(0.125 * scoresT) on ACT (PSUM->SBUF, f16), diag block masked
  outT[65,t] psum += v1_j^T @ wT_j    (row 64 accumulates the softmax denom)
  out[t,h] = PE-transpose(outT) / denom, stored f32
"""

import numpy as np

import concourse.bass as bass
import concourse.mybir as mybir
from concourse.tile import TileContext
from concourse.masks import make_identity, make_upper_triangular
from concourse.bass_utils import run_bass_kernel_spmd

B, T, E, H = 8, 2048, 1024, 64
NT = T // 128   # 16 t-tiles
NE = E // 128   # 8 e-tiles
F16 = mybir.dt.float16
F32 = mybir.dt.float32
SCALE = float(H) ** -0.5  # 0.125

def _split_excess_waits(nc: bass.Bass, cap: int = 1) -> int:
    """This walrus build allows only `cap` sync-waits per instruction
    ("Too many sync wait commands" in CoreV3GenImpl otherwise), while Tile's
    wait assignment attaches one wait per producer proc. Hoist the extra
    waits onto NOPs inserted just before the instruction on the same engine:
    the engine blocks on each NOP's wait first, so the semantics of
    "all waits hold before the instruction executes" are preserved."""
    n_split = 0
    for f in nc.m.functions:
        for bb in f.blocks:
            insts = list(bb.instructions)
            out = []
            dirty = False
            for inst in insts:
                si = inst.sync_info
                waits = list(si.on_wait) if si and si.on_wait else []
                if len(waits) > cap:
                    si.on_wait = waits[:cap]
                    for w in waits[cap:]:
                        nop = mybir.InstNoOp(
                            name=f"I-waitsplit-{n_split}", ins=[], outs=[]
                        )
                        nop.engine = inst.engine
                        nop.sync_info = mybir.SyncInfo(on_wait=[w], on_update=[])
                        out.append(nop)
                        n_split += 1
                    dirty = True
                out.append(inst)
            if dirty:
                bb.instructions = out
    return n_split


def build_nc(split_waits: bool = True) -> bass.Bass:
    nc = bass.Bass()
    x = nc.dram_tensor("x", [T, E], F32, kind="ExternalInput")
    wq = nc.dram_tensor("Wq", [E, H], F32, kind="ExternalInput")
    wk = nc.dram_tensor("Wk", [E, H], F32, kind="ExternalInput")
    wv = nc.dram_tensor("Wv", [E, H], F32, kind="ExternalInput")
    out = nc.dram_tensor("out", [T, H], F32, kind="ExternalOutput")
    x_ap, out_ap = x.ap(), out.ap()

    with TileContext(nc) as tc:
        with (
            tc.tile_pool(name="const", bufs=1) as cpool,
            tc.tile_pool(name="wts", bufs=8) as wpool,
            tc.tile_pool(name="xnat", bufs=8) as xnpool,
            tc.tile_pool(name="xT", bufs=8) as xtpool,
            tc.tile_pool(name="qkv", bufs=1) as qkvpool,
            tc.tile_pool(name="wT", bufs=3) as wtpool,
            tc.tile_pool(name="fin", bufs=4) as finpool,
        ):
            eye16 = cpool.tile([128, 128], F16, tag="eye16")
            make_identity(nc, eye16[:])
            eye32 = cpool.tile([128, 128], F32, tag="eye32")
            make_identity(nc, eye32[:])
            # tri[s,t] = 1 where s <= t (valid causal region of a diag block)
            tri = cpool.tile([128, 128], F16, tag="tri")
            make_upper_triangular(nc, tri[:], val=1.0, diag=True)

            w32 = {}
            for nm, dram in (("q", wq), ("k", wk), ("v", wv)):
                t32 = wpool.tile(
                    [128, NE * H], F32, tag="w32", bufs=3, name=f"w32{nm}"
                )
                nc.sync.dma_start(
                    t32[:].rearrange("p (j h) -> p j h", h=H),
                    dram.ap().rearrange("(j p) h -> p j h", p=128),
                )
                w32[nm] = t32
            wqk_t = []
            wv_t = []
            for j in range(NE):
                wt = wpool.tile([128, 128], F16, tag="wqk")
                nc.vector.tensor_copy(wt[:, 0:64], w32["q"][:, H * j : H * j + H])
                nc.vector.tensor_copy(wt[:, 64:128], w32["k"][:, H * j : H * j + H])
                wqk_t.append(wt)
                vt = wpool.tile([128, 64], F16, tag="wv")
                nc.vector.tensor_copy(vt[:], w32["v"][:, H * j : H * j + H])
                wv_t.append(vt)

            qkT = qkvpool.tile([128, T], F16, tag="qkT")  # q rows 0-63, k rows 64-127
            kT = qkvpool.tile([64, T], F16, tag="kT")
            vT = qkvpool.tile([64, T], F16, tag="vT")
            v1 = qkvpool.tile([128, NT * 65], F16, tag="v1")
            outT_sb = qkvpool.tile([65, T], F32, tag="outT")

            xT = [
                xtpool.tile([128, T], F16, tag="xT", name=f"xT{j}")
                for j in range(NE)
            ]

            # ---------- phase A: load x, transpose to xT, projections ----------
            with tc.tile_pool(name="ps1", bufs=1, space="PSUM") as ps1:
                xn = []
                for i in range(NT):
                    t_ = xnpool.tile([128, E], F16, tag="xn", bufs=10, name=f"xn{i}")
                    nc.gpsimd.dma_start(t_[:], x_ap[128 * i : 128 * i + 128, :])
                    xn.append(t_)

                # transpose 4 t-tiles at a time into one psum bank per e-tile
                for i0 in range(0, NT, 4):
                    for j in range(NE):
                        pt = ps1.tile([128, 512], F32, tag="tp", bufs=3)
                        for ii in range(4):
                            nc.tensor.matmul(
                                pt[:, 128 * ii : 128 * ii + 128],
                                xn[i0 + ii][:, 128 * j : 128 * j + 128],
                                eye16[:],
                                start=True, stop=True,
                            )
                        nc.scalar.copy(
                            xT[j][:, 128 * i0 : 128 * i0 + 512], pt[:]
                        )

                # projections: psum_qk[m, t] with q in rows 0-63, k in 64-127
                for c in range(4):
                    sl = slice(512 * c, 512 * c + 512)
                    pqk = ps1.tile([128, 512], F32, tag="pqk", bufs=2)
                    pv = ps1.tile([64, 512], F32, tag="pv", bufs=2)
                    for j in range(NE):
                        nc.tensor.matmul(
                            pqk[:], wqk_t[j][:], xT[j][:, sl],
                            start=(j == 0), stop=(j == NE - 1),
                        )
                        nc.tensor.matmul(
                            pv[:], wv_t[j][:], xT[j][:, sl],
                            start=(j == 0), stop=(j == NE - 1),
                        )
                    nc.vector.tensor_copy(qkT[:, sl], pqk[:])
                    nc.scalar.copy(vT[:, sl], pv[:])
                    # kT = k rows of qkT moved to partitions 0-63
                    nc.sync.dma_start(kT[:, sl], qkT[64:128, sl])

            # ---------- phase B: v1, attention, finalize ----------
            with tc.tile_pool(name="ps2", bufs=1, space="PSUM") as ps2:
                for i in range(NT):
                    pt = ps2.tile([128, 64], F32, tag="sc", bufs=2)
                    nc.tensor.matmul(
                        pt[:, 0:64], vT[:, 128 * i : 128 * i + 128],
                        eye16[0:64, 0:64], start=True, stop=True,
                    )
                    nc.vector.tensor_copy(v1[:, 65 * i : 65 * i + 64], pt[:, 0:64])
                    nc.gpsimd.memset(v1[:, 65 * i + 64 : 65 * i + 65], 1.0)

                outT_ps = [
                    ps2.tile([65, 512], F32, tag="ot", bufs=4, name=f"ot{c}")
                    for c in range(4)
                ]
                wTs = {}

                def emit_2ndmm(j):
                    # outT[65, t] += v1_j^T @ wT_j  per absolute-512 chunk
                    s0 = 128 * j
                    for c in range(s0 // 512, 4):
                        t0 = max(512 * c, s0)
                        t1 = 512 * c + 512
                        pad = t0 - 512 * c
                        nc.tensor.matmul(
                            outT_ps[c][:, pad:512],
                            v1[:, 65 * j : 65 * j + 65],
                            wTs[j][:, t0 - s0 : t1 - s0],
                            start=(j == 0), stop=(j == 4 * c + 3),
                        )

                for j in range(NT):
                    s0 = 128 * j  # this s-tile covers s in [s0, s0+128)
                    Lj = T - s0
                    wT = wtpool.tile([128, Lj], F16, tag="wT", bufs=4, name=f"wT{j}")
                    wTs[j] = wT
                    # scoresT in [128,<=1024] psum tiles on absolute-1024 grid
                    for kc in range(s0 // 1024, 2):
                        t0 = max(1024 * kc, s0)
                        t1 = 1024 * kc + 1024
                        w = t1 - t0
                        ps = ps2.tile([128, w], F32, tag="sc", bufs=2)
                        off = 0
                        while off < w:
                            n = min(512, w - off)
                            nc.tensor.matmul(
                                ps[:, off : off + n],
                                kT[:, s0 : s0 + 128],
                                qkT[0:64, t0 + off : t0 + off + n],
                                start=True, stop=True,
                            )
                            off += n
                        nc.scalar.activation(
                            wT[:, t0 - s0 : t1 - s0], ps[:],
                            mybir.ActivationFunctionType.Exp, scale=SCALE,
                        )
                    # mask the diagonal block: keep only s <= t
                    nc.vector.tensor_mul(wT[:, 0:128], wT[:, 0:128], tri[:])
                    # 2nd matmul lags one s-tile so PE never waits on exp_j
                    if j >= 1:
                        emit_2ndmm(j - 1)
                emit_2ndmm(NT - 1)
                for c in range(4):
                    nc.vector.tensor_copy(
                        outT_sb[:, 512 * c : 512 * c + 512], outT_ps[c][:]
                    )

                for i in range(NT):
                    pt = ps2.tile([128, 65], F32, tag="sc", bufs=2)
                    nc.tensor.transpose(
                        pt[:, 0:65],
                        outT_sb[:, 128 * i : 128 * i + 128],
                        eye32[0:65, 0:65],
                    )
                    rcp = finpool.tile([128, 1], F32, tag="rcp")
                    nc.vector.reciprocal(rcp[:], pt[:, 64:65])
                    ob = finpool.tile([128, 64], F32, tag="ob")
                    nc.vector.tensor_scalar_mul(ob[:], pt[:, 0:64], rcp[:])
                    nc.sync.dma_start(out_ap[128 * i : 128 * i + 128, :], ob[:])

    if split_waits:
        # required for walrus codegen; CoreSim rejects the raw NOPs, so
        # sim runs build with split_waits=False
        _split_excess_waits(nc)
    return nc


_NC = None


def _get_nc() -> bass.Bass:
    global _NC
    if _NC is None:
        _NC = build_nc()
    return _NC


def kernel(x, Wq, Wk, Wv, **run_kwargs):
    nc = _get_nc()
    x = np.ascontiguousarray(x, dtype=np.float32)
    in_maps = [
        {
            "x": np.ascontiguousarray(x[b]),
            "Wq": np.ascontiguousarray(Wq, dtype=np.float32),
            "Wk": np.ascontiguousarray(Wk, dtype=np.float32),
            "Wv": np.ascontiguousarray(Wv, dtype=np.float32),
        }
        for b in range(B)
    ]
    res = run_bass_kernel_spmd(nc, in_maps, core_ids=list(range(B)), **run_kwargs)
    out = np.stack([res.results[b]["out"] for b in range(B)], axis=0)
    kernel.last_results = res
    return out
